# revision 1
# baseline (speedup 1.0000x reference)
"""Causal self-attention (RoPE) Trainium2 Bass kernel, 8-way sharded.

Sharding: core c handles batch c//4 and heads 4*(c%4) .. 4*(c%4)+4
(tensor-parallel over heads x data-parallel over batch). Each core
computes its QKV column shard, RoPE, causal attention for its 4 heads,
and a row-shard of the out-projection; the host sums the 4 partial
outputs per batch (the all-reduce realized at gather time).

Per-core kernel layout (everything orientation-chosen to avoid on-chip
transposes):
  - x^T tiles [d,t] arrive pre-transposed from host.
  - Q^T,K^T [hd,t] = w_tile.T @ x^T  (w stationary), RoPE applied via a
    rotate-by-64 permutation-matrix matmul + aligned DVE combine.
  - V [t,hd] = x^T_tile.T @ w_v  (x^T stationary).
  - S^T [k,q] = K_slice.T @ Q_block ; exp on ACT (no max subtraction --
    scores are O(1) bounded); causal mask = multiplicative 0/1 post-exp.
  - softmax denominators via all-ones stationary matmul (broadcasts the
    partition-dim sums to all 128 partitions).
  - attn^T [hd,q] = V_tile.T @ P^T, normalized by reciprocal * mul.
  - out[t,e] = attnT_slice.T @ w_out_rows, accumulated over the 4 heads.
"""

import sys

if "/opt/trn_rl_repo" not in sys.path:
    sys.path.insert(0, "/opt/trn_rl_repo")

import numpy as np
import ml_dtypes

import concourse.bass as bass
import concourse.mybir as mybir
import concourse.tile as tile
from concourse import bacc
from concourse.bass_utils import run_bass_kernel_spmd

FP32 = mybir.dt.float32
BF16 = mybir.dt.bfloat16
BF16_NP = ml_dtypes.bfloat16

B = 2
T = 2048
DIM = 2048
NUM_HEADS = 16
HEAD_DIM = 128
INNER = NUM_HEADS * HEAD_DIM
N_CORES = 8
NH = 4            # heads per core
P = 128           # partitions
TB = T // 512     # 4 t-blocks of 512 tokens
DT = DIM // P     # 16 d-tiles
KT = T // P       # 16 k-tiles of 128 tokens
SCALE = 1.0 / float(np.sqrt(HEAD_DIM))

_CACHE = {}


def _build_nc(reps=1, opts=None):
    o = {
        "early_dma": True,    # load compute-critical tiles first
        "dt_major": True,     # QKV q/k matmuls dt-outer in groups of 4 c-tiles
        "ps_qk": 4,
        "ps_s": 3,
        "ps_r": 1,
        "ps_o": 2,
        "ps_y": 2,
        # ablation switches (timing accounting only -- break numerics)
        "no_sums": False,
        "no_rope": False,
        # rope via cross-partition DVE ops instead of perm matmul
        "rope_xpart": False,
        "qkv_only": False,
        "no_outproj": False,
        "no_exp": False,
        "spread_outproj": True,
        # sums grouping: DVE-add G pt tiles before each ones-matmul (1 = off)
        "sums_group": 4,
        # attention i-loop software-pipeline depth: S-matmuls emitted this
        # many tiles ahead of their sums/PV consumers, so PE has queued work
        # while ACT computes exp
        "lookahead": 2,
    }
    if opts:
        o.update(opts)
    nc = bacc.Bacc(None, target_bir_lowering=False)

    xt_d = nc.declare_dram_parameter("xt", [TB, P, DT * 512], BF16, isOutput=False)
    wqk_d = nc.declare_dram_parameter("wqk", [P, 8 * DT * P], BF16, isOutput=False)
    wv_d = nc.declare_dram_parameter("wv", [P, DT * 512], BF16, isOutput=False)
    wo_d = nc.declare_dram_parameter("wo", [P, NH * DIM], BF16, isOutput=False)
    cos_d = nc.declare_dram_parameter("cosT", [P, T], BF16, isOutput=False)
    sin_d = nc.declare_dram_parameter("sinT", [P, T], BF16, isOutput=False)
    mask_d = nc.declare_dram_parameter("mask", [P, 4 * 512], BF16, isOutput=False)
    perm_d = nc.declare_dram_parameter("perm", [P, P], BF16, isOutput=False)
    out_d = nc.declare_dram_parameter("out", [T, DIM], FP32, isOutput=True)

    EXP = mybir.ActivationFunctionType.Exp

    with tile.TileContext(nc) as tc:
        with (
            tc.tile_pool(name="const", bufs=1) as cpool,
            tc.tile_pool(name="qkstore", bufs=1) as qkpool,
            tc.tile_pool(name="vstore", bufs=1) as vpool,
            tc.tile_pool(name="xt", bufs=20) as xtpool,
            tc.tile_pool(name="tmp", bufs=3) as tmp,
            tc.tile_pool(name="pt", bufs=10) as ptpool,
            tc.tile_pool(name="ptm", bufs=6) as ptmpool,
            tc.tile_pool(name="attnT", bufs=8) as atpool,
            tc.tile_pool(name="outb", bufs=6) as outpool,
            tc.tile_pool(name="sacc", bufs=3) as saccpool,
        ):
            # --- constants ---
            wqk = cpool.tile([P, 8 * DT * P], BF16)
            wv = cpool.tile([P, DT * 512], BF16)
            wo = cpool.tile([P, NH * DIM], BF16)
            cosT = cpool.tile([P, T], BF16)
            sinT = cpool.tile([P, T], BF16)
            mask = cpool.tile([P, 4 * 512], BF16)
            perm = cpool.tile([P, P], BF16)
            ones = cpool.tile([P, P], BF16)

            def load_wqk_group(grp):
                # one ct-group = 4 c-tiles worth of stationary weight slices
                nc.sync.dma_start(
                    wqk[:, grp * 4 * DT * P : (grp + 1) * 4 * DT * P],
                    wqk_d[:, grp * 4 * DT * P : (grp + 1) * 4 * DT * P],
                )

            xt0 = []
            if o["early_dma"]:
                # compute-critical first: wqk group 0, xt(tb=0), rope tables
                load_wqk_group(0)
                for dt in range(DT):
                    xt_tile = xtpool.tile([P, 512], BF16, tag="xt")
                    nc.sync.dma_start(xt_tile[:], xt_d[0, :, dt * 512 : (dt + 1) * 512])
                    xt0.append(xt_tile)
                nc.sync.dma_start(perm[:], perm_d[:])
                nc.sync.dma_start(cosT[:], cos_d[:])
                nc.sync.dma_start(sinT[:], sin_d[:])
                load_wqk_group(1)
                nc.sync.dma_start(wv[:], wv_d[:])
                nc.sync.dma_start(mask[:], mask_d[:])
                nc.sync.dma_start(wo[:], wo_d[:])
            else:
                load_wqk_group(0)
                load_wqk_group(1)
                nc.sync.dma_start(wv[:], wv_d[:])
                nc.sync.dma_start(wo[:], wo_d[:])
                nc.sync.dma_start(cosT[:], cos_d[:])
                nc.sync.dma_start(sinT[:], sin_d[:])
                nc.sync.dma_start(mask[:], mask_d[:])
                nc.sync.dma_start(perm[:], perm_d[:])
            nc.gpsimd.memset(ones[:], 1.0)

            # persistent stores: Q^T,K^T post-rope [hd, T] per (q/k, head);
            # V [t-tile-major, hd] per head
            qkstore = qkpool.tile([P, 8 * T], BF16)   # ct = (q h0..h3, k h0..h3)
            vstore = vpool.tile([P, NH * T], BF16)    # per head: (kt, hd)

            # ---------------- QKV + RoPE phase ----------------
            for _rep in range(reps):
              with (
                tc.tile_pool(name="ps_qk", bufs=o["ps_qk"], space="PSUM") as ps_qk,
                tc.tile_pool(name="ps_v", bufs=2, space="PSUM") as ps_v,
                tc.tile_pool(name="ps_rope", bufs=2, space="PSUM") as ps_rope,
              ):
                for tb in range(TB):
                    if tb == 0 and o["early_dma"] and _rep == 0:
                        xt_t = xt0
                    else:
                        xt_t = []
                        for dt in range(DT):
                            xt_tile = xtpool.tile([P, 512], BF16, tag="xt")
                            nc.sync.dma_start(
                                xt_tile[:], xt_d[tb, :, dt * 512 : (dt + 1) * 512]
                            )
                            xt_t.append(xt_tile)

                    def rope_and_store(ps, ct):
                        if o["no_rope"]:
                            nc.scalar.copy(
                                qkstore[:, ct * T + tb * 512 : ct * T + (tb + 1) * 512],
                                ps[:],
                            )
                            return
                        t1 = tmp.tile([P, 512], FP32, tag="t1")
                        nc.vector.tensor_mul(
                            t1[:], ps[:], cosT[:, tb * 512 : (tb + 1) * 512]
                        )
                        t2 = tmp.tile([P, 512], FP32, tag="t2")
                        tbs = slice(tb * 512, (tb + 1) * 512)
                        if o["rope_xpart"]:
                            nc.vector.tensor_mul(
                                t2[0:64, :], ps[64:128, :], sinT[0:64, tbs]
                            )
                            nc.vector.tensor_mul(
                                t2[64:128, :], ps[0:64, :], sinT[64:128, tbs]
                            )
                        else:
                            qsb = tmp.tile([P, 512], BF16, tag="qsb")
                            nc.scalar.copy(qsb[:], ps[:])
                            sw = ps_rope.tile([P, 512], FP32)
                            nc.tensor.matmul(
                                sw[:], perm[:], qsb[:], start=True, stop=True
                            )
                            nc.vector.tensor_mul(t2[:], sw[:], sinT[:, tbs])
                        nc.vector.tensor_add(
                            qkstore[:, ct * T + tb * 512 : ct * T + (tb + 1) * 512],
                            t1[:],
                            t2[:],
                        )

                    # Q^T, K^T c-tiles with RoPE
                    if o["dt_major"]:
                        for grp in range(2):
                            pss = [ps_qk.tile([P, 512], FP32, name="psqk", tag="psqk") for _ in range(4)]
                            for dt in range(DT):
                                for ci in range(4):
                                    ct = grp * 4 + ci
                                    nc.tensor.matmul(
                                        pss[ci][:],
                                        wqk[:, (ct * DT + dt) * P : (ct * DT + dt + 1) * P],
                                        xt_t[dt][:],
                                        start=(dt == 0),
                                        stop=(dt == DT - 1),
                                    )
                            for ci in range(4):
                                rope_and_store(pss[ci], grp * 4 + ci)
                    else:
                        for ct in range(8):
                            ps = ps_qk.tile([P, 512], FP32)
                            for dt in range(DT):
                                nc.tensor.matmul(
                                    ps[:],
                                    wqk[:, (ct * DT + dt) * P : (ct * DT + dt + 1) * P],
                                    xt_t[dt][:],
                                    start=(dt == 0),
                                    stop=(dt == DT - 1),
                                )
                            rope_and_store(ps, ct)

                    # V tiles [t, c] for 4 heads
                    for s in range(4):
                        psv = ps_v.tile([P, 512], FP32)
                        for dt in range(DT):
                            nc.tensor.matmul(
                                psv[:],
                                xt_t[dt][:, s * P : (s + 1) * P],
                                wv[:, dt * 512 : (dt + 1) * 512],
                                start=(dt == 0),
                                stop=(dt == DT - 1),
                            )
                        kt_idx = tb * 4 + s
                        for h in range(NH):
                            nc.vector.tensor_copy(
                                vstore[:, h * T + kt_idx * P : h * T + (kt_idx + 1) * P],
                                psv[:, h * P : (h + 1) * P],
                            )

              # ---------------- attention + out-proj phase ----------------
              if o["qkv_only"]:
                  continue
              with (
                tc.tile_pool(name="ps_s", bufs=o["ps_s"], space="PSUM") as ps_s,
                tc.tile_pool(name="ps_r", bufs=o["ps_r"], space="PSUM") as ps_r,
                tc.tile_pool(name="ps_o", bufs=o["ps_o"], space="PSUM") as ps_o,
                tc.tile_pool(name="ps_y", bufs=o["ps_y"], space="PSUM") as ps_y,
              ):
                # Global software pipeline across (j, h, i): the S-matmul/exp
                # producer cursor runs `lookahead` stages ahead of the
                # sums/PV consumer cursor, so PE always has independent
                # S-matmuls queued while ACT computes exp. Out-proj emits as
                # soon as its j's consumers have drained, filling PE while
                # ACT works on the next j's exps.
                LA = o["lookahead"]
                pts = {}       # (j,h,i) -> pt tile
                ros = {}       # (j,h) -> (r_ps, o_ps)
                at_tiles = {}  # (j,h) -> at

                stages = [
                    (j, h, i)
                    for j in range(TB)
                    for h in range(NH)
                    for i in range(4 * j + 4)
                ]

                def emit_s(key):
                    j, h, i = key
                    qoff = h * T
                    koff = (NH + h) * T
                    s_ps = ps_s.tile([P, 512], FP32, name="s_ps", tag="s_ps")
                    nc.tensor.matmul(
                        s_ps[:],
                        qkstore[:, koff + i * P : koff + (i + 1) * P],
                        qkstore[:, qoff + j * 512 : qoff + (j + 1) * 512],
                        start=True,
                        stop=True,
                    )
                    if o["no_exp"]:
                        pts[key] = mask[:, 0:512]
                        return
                    pt = ptpool.tile([P, 512], BF16, name="pt", tag="pt")
                    nc.scalar.activation(pt[:], s_ps[:], EXP, scale=SCALE)
                    if i >= 4 * j:
                        ptm = ptmpool.tile([P, 512], BF16, name="ptm", tag="ptm")
                        off = i - 4 * j
                        nc.vector.tensor_mul(
                            ptm[:], pt[:], mask[:, off * 512 : (off + 1) * 512]
                        )
                        pt = ptm
                    pts[key] = pt

                sum_pend = {}

                def emit_consume(key):
                    j, h, i = key
                    n_i = 4 * j + 4
                    G = o["sums_group"]
                    pt = pts.pop(key)
                    if i == 0:
                        o_ps = ps_o.tile([P, 512], FP32, name="o_ps", tag="o_ps")
                        r_ps = (
                            None
                            if o["no_sums"]
                            else ps_r.tile([P, 512], FP32, name="r_ps", tag="r_ps")
                        )
                        ros[(j, h)] = (r_ps, o_ps)
                        sum_pend[(j, h)] = ([], [0])
                    r_ps, o_ps = ros[(j, h)]
                    nc.tensor.matmul(
                        o_ps[:],
                        vstore[:, h * T + i * P : h * T + (i + 1) * P],
                        pt[:],
                        start=(i == 0), stop=(i == n_i - 1),
                    )
                    if not o["no_sums"]:
                        pend, gidx = sum_pend[(j, h)]
                        pend.append(pt)
                        if len(pend) == G or i == n_i - 1:
                            if len(pend) == 1:
                                rhs = pend[0]
                            else:
                                acc = saccpool.tile(
                                    [P, 512], BF16, name="sacc", tag="sacc"
                                )
                                nc.vector.tensor_add(acc[:], pend[0][:], pend[1][:])
                                for extra in pend[2:]:
                                    nc.vector.tensor_add(acc[:], acc[:], extra[:])
                                rhs = acc
                            nc.tensor.matmul(
                                r_ps[:], ones[:], rhs[:],
                                start=(gidx[0] == 0), stop=(i == n_i - 1),
                            )
                            pend.clear()
                            gidx[0] += 1
                    if i == n_i - 1:
                        r_ps, o_ps = ros.pop((j, h))
                        at = atpool.tile([P, 512], BF16, name="at", tag="at")
                        if o["no_sums"]:
                            nc.vector.tensor_copy(at[:], o_ps[:])
                        else:
                            rc = tmp.tile([P, 512], FP32, tag="rc")
                            nc.vector.reciprocal(rc[:], r_ps[:])
                            nc.vector.tensor_mul(at[:], o_ps[:], rc[:])
                        at_tiles[(j, h)] = at
                        if h == NH - 1:
                            if o["no_outproj"]:
                                for hh in range(NH):
                                    at_tiles.pop((j, hh))
                            else:
                                emit_outproj(j)

                y_pend = []

                def emit_y_group(j, at_j, s, e):
                    y_ps = ps_y.tile([P, 512], FP32, name="y_ps", tag="y_ps")
                    for h in range(NH):
                        nc.tensor.matmul(
                            y_ps[:],
                            at_j[h][:, s * P : (s + 1) * P],
                            wo[:, h * DIM + e * 512 : h * DIM + (e + 1) * 512],
                            start=(h == 0),
                            stop=(h == NH - 1),
                        )
                    yo = outpool.tile([P, 512], FP32, tag="yo")
                    nc.vector.tensor_copy(yo[:], y_ps[:])
                    t0 = j * 512 + s * P
                    nc.sync.dma_start(
                        out_d[t0 : t0 + P, e * 512 : (e + 1) * 512], yo[:]
                    )

                def emit_outproj(j):
                    at_j = [at_tiles.pop((j, h)) for h in range(NH)]
                    groups = [(j, at_j, s, e) for s in range(4) for e in range(4)]
                    if o["spread_outproj"]:
                        y_pend.extend(groups)
                    else:
                        for g in groups:
                            emit_y_group(*g)

                for k in range(len(stages) + LA):
                    if k < len(stages):
                        emit_s(stages[k])
                    if k - LA >= 0:
                        emit_consume(stages[k - LA])
                    if y_pend:
                        emit_y_group(*y_pend.pop(0))
                while y_pend:
                    emit_y_group(*y_pend.pop(0))

    nc.compile()
    return nc


def _rope_tables():
    inv_freq = 1.0 / (
        10000.0 ** (np.arange(0, HEAD_DIM, 2, dtype=np.float32) / HEAD_DIM)
    )
    t = np.arange(T, dtype=np.float32)
    freqs = np.einsum("i,j->ij", t, inv_freq)          # [T, 64]
    emb = np.concatenate([freqs, freqs], axis=-1)      # [T, 128]
    cosT = np.cos(emb).T.astype(BF16_NP)               # [128, T]
    sinT = np.sin(emb).T                               # [128, T]
    sinS = np.concatenate([-sinT[:64], sinT[64:]], axis=0).astype(BF16_NP)
    return np.ascontiguousarray(cosT), np.ascontiguousarray(sinS)


def kernel(x, w_qkv, w_out):
    x = np.asarray(x, dtype=np.float32)
    w_qkv = np.asarray(w_qkv, dtype=np.float32)
    w_out = np.asarray(w_out, dtype=np.float32)

    cosT, sinS = _rope_tables()

    perm = np.zeros((P, P), dtype=BF16_NP)
    for i in range(P):
        perm[(i + 64) % P, i] = 1

    mask = np.zeros((P, 4 * 512), dtype=BF16_NP)
    r_idx = np.arange(P)[:, None]
    c_idx = np.arange(512)[None, :]
    for oi, off in enumerate((0, 128, 256, 384)):
        mask[:, oi * 512 : (oi + 1) * 512] = (r_idx + off <= c_idx).astype(BF16_NP)

    # per-batch x^T tiles: [TB, 128, DT*512]
    xts = []
    for b in range(B):
        xT = np.ascontiguousarray(x[b].T).astype(BF16_NP)          # [D, T]
        xth = (
            xT.reshape(DT, P, TB, 512).transpose(2, 1, 0, 3).reshape(TB, P, DT * 512)
        )
        xts.append(np.ascontiguousarray(xth))

    # per head-group weight shards
    wqks, wvs, wos = [], [], []
    for g in range(4):
        h0 = NH * g
        cols = [w_qkv[:, 128 * (h0 + h) : 128 * (h0 + h + 1)] for h in range(NH)]
        cols += [
            w_qkv[:, INNER + 128 * (h0 + h) : INNER + 128 * (h0 + h + 1)]
            for h in range(NH)
        ]
        W = np.concatenate(cols, axis=1)                            # [D, 8*128]
        wqk_h = (
            W.reshape(DT, P, 8, P).transpose(1, 2, 0, 3).reshape(P, 8 * DT * P)
        ).astype(BF16_NP)
        wqks.append(np.ascontiguousarray(wqk_h))

        WV = w_qkv[:, 2 * INNER + 128 * h0 : 2 * INNER + 128 * (h0 + NH)]  # [D, 512]
        wv_h = WV.reshape(DT, P, 512).transpose(1, 0, 2).reshape(P, DT * 512)
        wvs.append(np.ascontiguousarray(wv_h.astype(BF16_NP)))

        WO = w_out[128 * h0 : 128 * (h0 + NH), :]                   # [512, D]
        wo_h = WO.reshape(NH, P, DIM).transpose(1, 0, 2).reshape(P, NH * DIM)
        wos.append(np.ascontiguousarray(wo_h.astype(BF16_NP)))

    if "nc" not in _CACHE:
        _CACHE["nc"] = _build_nc()
    nc = _CACHE["nc"]

    in_maps = []
    for c in range(N_CORES):
        b, g = divmod(c, 4)
        in_maps.append(
            {
                "xt": xts[b],
                "wqk": wqks[g],
                "wv": wvs[g],
                "wo": wos[g],
                "cosT": cosT,
                "sinT": sinS,
                "mask": mask,
                "perm": perm,
            }
        )

    res = run_bass_kernel_spmd(nc, in_maps, core_ids=list(range(N_CORES)))

    out = np.zeros((B, T, DIM), dtype=np.float32)
    for c in range(N_CORES):
        b = c // 4
        out[b] += res.results[c]["out"]
    return out



# revision 3
# speedup vs baseline: 1.0758x; 1.0758x over previous
"""Causal self-attention (RoPE) Trainium2 Bass kernel, 8-way sharded.

Sharding: core c handles batch c//4 and heads 4*(c%4) .. 4*(c%4)+4
(tensor-parallel over heads x data-parallel over batch). Each core
computes its QKV column shard, RoPE, causal attention for its 4 heads,
and a row-shard of the out-projection; the host sums the 4 partial
outputs per batch (the all-reduce realized at gather time).

Matmul precision scheme: the three dense projections (QKV, V, out-proj)
run as fp8e4 DoubleRow "3-term" products: each operand is pre-scaled by
a power of two and split into an e4m3 hi+lo pair (hi = fp8(x), lo =
fp8(x - hi), both at the same scale so one PSUM group accumulates all
terms); the product keeps XhWh + XlWh + XhWl and drops the O(eps^2)
XlWl term. DoubleRow pairs two 128-deep contraction slots per
instruction at 0.5 cycles/row, so the 3-term product costs 0.75x a
bf16 matmul with ~bf16 accuracy. The 2^-11 descale is folded into the
constant tables / PSUM-read copies. Attention (S, exp, PV, sums) stays
bf16: its 128-deep contractions cannot amortize the extra DoubleRow
slots without a full-rate plain-fp8 factor, which costs ~2.5e-2 rel
error (measured) vs the 2e-2 gate.

Per-core kernel layout (orientation-chosen to avoid on-chip
transposes):
  - x^T tiles [d,t] arrive as pre-scaled fp8 hi/lo pairs from host.
  - Q^T,K^T [hd,t] = w.T @ x^T (w stationary) via 3-term DoubleRow,
    RoPE applied via a rotate-by-64 permutation-matrix matmul +
    aligned DVE combine.
  - V [t,hd] = x^T_tile.T @ w_v (x^T stationary), 3-term DoubleRow.
  - S^T [k,q] = K_slice.T @ Q_block ; exp on ACT (no max subtraction --
    scores are O(1) bounded); causal mask = multiplicative 0/1 post-exp.
  - softmax denominators via (1/32)-constant stationary matmul
    (broadcasts the partition-dim sums to all 128 partitions).
  - attn^T [hd,q] = V_tile.T @ P^T; at32 = 32*attn via reciprocal*mul,
    split into fp8 hi/lo (ACT copy + DVE scalar_tensor_tensor).
  - out[t,e] = at_slice.T @ w_out_rows via 3-term DoubleRow over
    head pairs, output written bf16 (host accumulates in fp32).
"""

import sys

if "/opt/trn_rl_repo" not in sys.path:
    sys.path.insert(0, "/opt/trn_rl_repo")

import numpy as np
import ml_dtypes

import concourse.bass as bass
import concourse.mybir as mybir
import concourse.tile as tile
from concourse import bacc
from concourse.bass_utils import run_bass_kernel_spmd

FP32 = mybir.dt.float32
BF16 = mybir.dt.bfloat16
FP8 = mybir.dt.float8e4
BF16_NP = ml_dtypes.bfloat16
F8_NP = ml_dtypes.float8_e4m3
DR = mybir.MatmulPerfMode.DoubleRow

B = 2
T = 2048
DIM = 2048
NUM_HEADS = 16
HEAD_DIM = 128
INNER = NUM_HEADS * HEAD_DIM
N_CORES = 8
NH = 4            # heads per core
P = 128           # partitions
TB = T // 512     # 4 t-blocks of 512 tokens
DT = DIM // P     # 16 d-tiles
KT = T // P       # 16 k-tiles of 128 tokens
SCALE = 1.0 / float(np.sqrt(HEAD_DIM))

SX = 32.0         # fp8 pre-scale for x^T and at
SW = 64.0         # fp8 pre-scale for weights
DESCALE = 1.0 / (SX * SW)   # 1/2048 folded into PSUM reads
RDEN = 32.0       # sums matmul constant = 1/RDEN; rc = RDEN/denom

_CACHE = {}


def _build_nc(reps=1, opts=None):
    o = {
        "early_dma": True,
        "ps_qk": 4,
        "ps_s": 3,
        "ps_r": 1,
        "ps_o": 2,
        "ps_y": 2,
        "sums_group": 8,
        "lookahead": 2,
        "spread_outproj": True,
    }
    if opts:
        o.update(opts)
    nc = bacc.Bacc(None, target_bir_lowering=False)

    xh_d = nc.declare_dram_parameter("xh", [TB, P, DT, 512], FP8, isOutput=False)
    xl_d = nc.declare_dram_parameter("xl", [TB, P, DT, 512], FP8, isOutput=False)
    wqkh_d = nc.declare_dram_parameter("wqkh", [P, 8 * DT, P], FP8, isOutput=False)
    wqkl_d = nc.declare_dram_parameter("wqkl", [P, 8 * DT, P], FP8, isOutput=False)
    wvh_d = nc.declare_dram_parameter("wvh", [P, DT, 512], FP8, isOutput=False)
    wvl_d = nc.declare_dram_parameter("wvl", [P, DT, 512], FP8, isOutput=False)
    woh_d = nc.declare_dram_parameter("woh", [P, NH, DIM], FP8, isOutput=False)
    wol_d = nc.declare_dram_parameter("wol", [P, NH, DIM], FP8, isOutput=False)
    cos_d = nc.declare_dram_parameter("cosT", [P, T], BF16, isOutput=False)
    sin_d = nc.declare_dram_parameter("sinT", [P, T], BF16, isOutput=False)
    mask_d = nc.declare_dram_parameter("mask", [P, 4 * 512], BF16, isOutput=False)
    perm_d = nc.declare_dram_parameter("perm", [P, P], BF16, isOutput=False)
    out_d = nc.declare_dram_parameter("out", [T, DIM], BF16, isOutput=True)

    EXP = mybir.ActivationFunctionType.Exp

    with tile.TileContext(nc) as tc:
        with (
            tc.tile_pool(name="const", bufs=1) as cpool,
            tc.tile_pool(name="qkstore", bufs=1) as qkpool,
            tc.tile_pool(name="vstore", bufs=1) as vpool,
            tc.tile_pool(name="xt", bufs=2) as xtpool,
            tc.tile_pool(name="tmp", bufs=2) as tmp,
            tc.tile_pool(name="pt", bufs=8) as ptpool,
            tc.tile_pool(name="ptm", bufs=4) as ptmpool,
            tc.tile_pool(name="attnT", bufs=2) as atpool,
            tc.tile_pool(name="outb", bufs=4) as outpool,
            tc.tile_pool(name="sacc", bufs=2) as saccpool,
        ):
            # --- constants ---
            wqkh = cpool.tile([P, 8 * DT, P], FP8)
            wqkl = cpool.tile([P, 8 * DT, P], FP8)
            wvh = cpool.tile([P, DT, 512], FP8)
            wvl = cpool.tile([P, DT, 512], FP8)
            woh = cpool.tile([P, NH, DIM], FP8)
            wol = cpool.tile([P, NH, DIM], FP8)
            cosT = cpool.tile([P, T], BF16)
            sinT = cpool.tile([P, T], BF16)
            mask = cpool.tile([P, 4 * 512], BF16)
            perm = cpool.tile([P, P], BF16)
            ones = cpool.tile([P, P], BF16)

            xt0 = None
            if o["early_dma"]:
                # compute-critical first: wqk hi, xt(tb=0), rope tables
                nc.sync.dma_start(wqkh[:], wqkh_d[:])
                xh0 = xtpool.tile([P, DT, 512], FP8, tag="xh")
                xl0 = xtpool.tile([P, DT, 512], FP8, tag="xl")
                nc.sync.dma_start(xh0[:], xh_d[0])
                nc.sync.dma_start(xl0[:], xl_d[0])
                xt0 = (xh0, xl0)
                nc.sync.dma_start(wqkl[:], wqkl_d[:])
                nc.sync.dma_start(perm[:], perm_d[:])
                nc.sync.dma_start(cosT[:], cos_d[:])
                nc.sync.dma_start(sinT[:], sin_d[:])
                nc.sync.dma_start(wvh[:], wvh_d[:])
                nc.sync.dma_start(wvl[:], wvl_d[:])
                nc.sync.dma_start(mask[:], mask_d[:])
                nc.sync.dma_start(woh[:], woh_d[:])
                nc.sync.dma_start(wol[:], wol_d[:])
            else:
                for t_, d_ in (
                    (wqkh, wqkh_d), (wqkl, wqkl_d), (wvh, wvh_d), (wvl, wvl_d),
                    (woh, woh_d), (wol, wol_d), (cosT, cos_d), (sinT, sin_d),
                    (mask, mask_d), (perm, perm_d),
                ):
                    nc.sync.dma_start(t_[:], d_[:])
            nc.gpsimd.memset(ones[:], 1.0 / RDEN)

            # persistent stores: Q^T,K^T post-rope [hd, T] per (q/k, head);
            # V [t-tile-major, hd] per head
            qkstore = qkpool.tile([P, 8 * T], BF16)   # ct = (q h0..h3, k h0..h3)
            vstore = vpool.tile([P, NH * T], BF16)    # per head: (kt, hd)

            # ---------------- QKV + RoPE phase ----------------
            for _rep in range(reps):
              with (
                tc.tile_pool(name="ps_qk", bufs=o["ps_qk"], space="PSUM") as ps_qk,
                tc.tile_pool(name="ps_v", bufs=2, space="PSUM") as ps_v,
                tc.tile_pool(name="ps_rope", bufs=2, space="PSUM") as ps_rope,
              ):
                for tb in range(TB):
                    if tb == 0 and o["early_dma"] and _rep == 0:
                        xh_t, xl_t = xt0
                    else:
                        xh_t = xtpool.tile([P, DT, 512], FP8, tag="xh")
                        xl_t = xtpool.tile([P, DT, 512], FP8, tag="xl")
                        nc.sync.dma_start(xh_t[:], xh_d[tb])
                        nc.sync.dma_start(xl_t[:], xl_d[tb])

                    def rope_and_store(ps, ct):
                        t1 = tmp.tile([P, 512], FP32, tag="t1")
                        tbs = slice(tb * 512, (tb + 1) * 512)
                        nc.vector.tensor_mul(t1[:], ps[:], cosT[:, tbs])
                        qsb = tmp.tile([P, 512], BF16, tag="qsb")
                        nc.scalar.mul(qsb[:], ps[:], DESCALE)
                        sw = ps_rope.tile([P, 512], FP32)
                        nc.tensor.matmul(sw[:], perm[:], qsb[:], start=True, stop=True)
                        t2 = tmp.tile([P, 512], FP32, tag="t2")
                        nc.vector.tensor_mul(t2[:], sw[:], sinT[:, tbs])
                        nc.vector.tensor_add(
                            qkstore[:, ct * T + tb * 512 : ct * T + (tb + 1) * 512],
                            t1[:],
                            t2[:],
                        )

                    # Q^T, K^T c-tiles with RoPE: 3-term DoubleRow over
                    # 8 dt-pairs, 4 interleaved psum groups
                    for grp in range(2):
                        pss = [
                            ps_qk.tile([P, 512], FP32, name="psqk", tag="psqk")
                            for _ in range(4)
                        ]
                        for pr in range(DT // 2):
                            d0 = 2 * pr
                            for ci in range(4):
                                ct = grp * 4 + ci
                                w0 = ct * DT + d0
                                for term in range(3):
                                    lhs = (wqkh if term != 1 else wqkl)[
                                        :, w0 : w0 + 2, :
                                    ]
                                    rhs = (xh_t if term != 2 else xl_t)[
                                        :, d0 : d0 + 2, :
                                    ]
                                    nc.tensor.matmul(
                                        pss[ci][:],
                                        lhs,
                                        rhs,
                                        start=(pr == 0 and term == 0),
                                        stop=(pr == DT // 2 - 1 and term == 2),
                                        perf_mode=DR,
                                    )
                        for ci in range(4):
                            rope_and_store(pss[ci], grp * 4 + ci)

                    # V tiles [t, c] for 4 heads: 3-term DoubleRow
                    for s in range(4):
                        psv = ps_v.tile([P, 512], FP32)
                        for pr in range(DT // 2):
                            d0 = 2 * pr
                            for term in range(3):
                                lhs = (xh_t if term != 1 else xl_t)[
                                    :, d0 : d0 + 2, s * P : (s + 1) * P
                                ]
                                rhs = (wvh if term != 2 else wvl)[:, d0 : d0 + 2, :]
                                nc.tensor.matmul(
                                    psv[:],
                                    lhs,
                                    rhs,
                                    start=(pr == 0 and term == 0),
                                    stop=(pr == DT // 2 - 1 and term == 2),
                                    perf_mode=DR,
                                )
                        kt_idx = tb * 4 + s
                        for h in range(NH):
                            nc.scalar.mul(
                                vstore[:, h * T + kt_idx * P : h * T + (kt_idx + 1) * P],
                                psv[:, h * P : (h + 1) * P],
                                DESCALE,
                            )

              # ---------------- attention + out-proj phase ----------------
              with (
                tc.tile_pool(name="ps_s", bufs=o["ps_s"], space="PSUM") as ps_s,
                tc.tile_pool(name="ps_r", bufs=o["ps_r"], space="PSUM") as ps_r,
                tc.tile_pool(name="ps_o", bufs=o["ps_o"], space="PSUM") as ps_o,
                tc.tile_pool(name="ps_y", bufs=o["ps_y"], space="PSUM") as ps_y,
              ):
                # Global software pipeline across (j, h, i): the S-matmul/exp
                # producer cursor runs `lookahead` stages ahead of the
                # sums/PV consumer cursor, so PE always has independent
                # S-matmuls queued while ACT computes exp. Out-proj emits as
                # soon as its j's consumers have drained.
                LA = o["lookahead"]
                pts = {}       # (j,h,i) -> pt tile
                ros = {}       # (j,h) -> (r_ps, o_ps)
                at_tiles = {}  # j -> (ath, atl)

                stages = [
                    (j, h, i)
                    for j in range(TB)
                    for h in range(NH)
                    for i in range(4 * j + 4)
                ]

                def emit_s(key):
                    j, h, i = key
                    qoff = h * T
                    koff = (NH + h) * T
                    s_ps = ps_s.tile([P, 512], FP32, name="s_ps", tag="s_ps")
                    nc.tensor.matmul(
                        s_ps[:],
                        qkstore[:, koff + i * P : koff + (i + 1) * P],
                        qkstore[:, qoff + j * 512 : qoff + (j + 1) * 512],
                        start=True,
                        stop=True,
                    )
                    pt = ptpool.tile([P, 512], BF16, name="pt", tag="pt")
                    nc.scalar.activation(pt[:], s_ps[:], EXP, scale=SCALE)
                    if i >= 4 * j:
                        ptm = ptmpool.tile([P, 512], BF16, name="ptm", tag="ptm")
                        off = i - 4 * j
                        nc.vector.tensor_mul(
                            ptm[:], pt[:], mask[:, off * 512 : (off + 1) * 512]
                        )
                        pt = ptm
                    pts[key] = pt

                sum_pend = {}

                def emit_consume(key):
                    j, h, i = key
                    n_i = 4 * j + 4
                    G = o["sums_group"]
                    pt = pts.pop(key)
                    if i == 0:
                        o_ps = ps_o.tile([P, 512], FP32, name="o_ps", tag="o_ps")
                        r_ps = ps_r.tile([P, 512], FP32, name="r_ps", tag="r_ps")
                        ros[(j, h)] = (r_ps, o_ps)
                        sum_pend[(j, h)] = ([], [0])
                    r_ps, o_ps = ros[(j, h)]
                    nc.tensor.matmul(
                        o_ps[:],
                        vstore[:, h * T + i * P : h * T + (i + 1) * P],
                        pt[:],
                        start=(i == 0), stop=(i == n_i - 1),
                    )
                    pend, gidx = sum_pend[(j, h)]
                    pend.append(pt)
                    if len(pend) == G or i == n_i - 1:
                        if len(pend) == 1:
                            rhs = pend[0]
                        else:
                            acc = saccpool.tile(
                                [P, 512], BF16, name="sacc", tag="sacc"
                            )
                            nc.vector.tensor_add(acc[:], pend[0][:], pend[1][:])
                            for extra in pend[2:]:
                                nc.vector.tensor_add(acc[:], acc[:], extra[:])
                            rhs = acc
                        nc.tensor.matmul(
                            r_ps[:], ones[:], rhs[:],
                            start=(gidx[0] == 0), stop=(i == n_i - 1),
                        )
                        pend.clear()
                        gidx[0] += 1
                    if i == n_i - 1:
                        r_ps, o_ps = ros.pop((j, h))
                        if h == 0:
                            ath = atpool.tile([P, NH, 512], FP8, name="ath", tag="ath")
                            atl = atpool.tile([P, NH, 512], FP8, name="atl", tag="atl")
                            at_tiles[j] = (ath, atl)
                        ath, atl = at_tiles[j]
                        rc = tmp.tile([P, 512], FP32, tag="rc")
                        nc.vector.reciprocal(rc[:], r_ps[:])
                        at32 = tmp.tile([P, 512], BF16, tag="at32")
                        nc.vector.tensor_mul(at32[:], o_ps[:], rc[:])
                        nc.scalar.copy(ath[:, h, :], at32[:])
                        nc.vector.scalar_tensor_tensor(
                            atl[:, h, :],
                            ath[:, h, :],
                            -1.0,
                            at32[:],
                            mybir.AluOpType.mult,
                            mybir.AluOpType.add,
                        )
                        if h == NH - 1:
                            emit_outproj(j)

                y_pend = []

                def emit_y_group(j, ath, atl, s, e):
                    y_ps = ps_y.tile([P, 512], FP32, name="y_ps", tag="y_ps")
                    ss = slice(s * P, (s + 1) * P)
                    es = slice(e * 512, (e + 1) * 512)
                    first, last = (0, 0), (1, 2)
                    for hp in range(2):
                        h0 = 2 * hp
                        for term in range(3):
                            lhs = (ath if term != 1 else atl)[:, h0 : h0 + 2, ss]
                            rhs = (woh if term != 2 else wol)[:, h0 : h0 + 2, es]
                            nc.tensor.matmul(
                                y_ps[:],
                                lhs,
                                rhs,
                                start=((hp, term) == first),
                                stop=((hp, term) == last),
                                perf_mode=DR,
                            )
                    yo = outpool.tile([P, 512], BF16, tag="yo")
                    nc.scalar.mul(yo[:], y_ps[:], DESCALE)
                    t0 = j * 512 + s * P
                    nc.sync.dma_start(
                        out_d[t0 : t0 + P, e * 512 : (e + 1) * 512], yo[:]
                    )

                def emit_outproj(j):
                    ath, atl = at_tiles.pop(j)
                    groups = [(j, ath, atl, s, e) for s in range(4) for e in range(4)]
                    if o["spread_outproj"]:
                        y_pend.extend(groups)
                    else:
                        for g in groups:
                            emit_y_group(*g)

                for k in range(len(stages) + LA):
                    if k < len(stages):
                        emit_s(stages[k])
                    if k - LA >= 0:
                        emit_consume(stages[k - LA])
                    if y_pend:
                        emit_y_group(*y_pend.pop(0))
                while y_pend:
                    emit_y_group(*y_pend.pop(0))

    nc.compile()
    return nc


def _rope_tables():
    inv_freq = 1.0 / (
        10000.0 ** (np.arange(0, HEAD_DIM, 2, dtype=np.float32) / HEAD_DIM)
    )
    t = np.arange(T, dtype=np.float32)
    freqs = np.einsum("i,j->ij", t, inv_freq)          # [T, 64]
    emb = np.concatenate([freqs, freqs], axis=-1)      # [T, 128]
    # cos table pre-descaled by 1/(SX*SW): it multiplies the scaled PSUM
    cosT = (np.cos(emb).T * DESCALE).astype(BF16_NP)   # [128, T]
    sinT = np.sin(emb).T                               # [128, T]
    sinS = np.concatenate([-sinT[:64], sinT[64:]], axis=0).astype(BF16_NP)
    return np.ascontiguousarray(cosT), np.ascontiguousarray(sinS)


def _split8(a, scale):
    """a*scale -> (hi, lo) fp8e4 pair at the same scale."""
    s = np.asarray(a, np.float32) * scale
    hi = s.astype(F8_NP)
    lo = (s - hi.astype(np.float32)).astype(F8_NP)
    return hi, lo


def kernel(x, w_qkv, w_out):
    x = np.asarray(x, dtype=np.float32)
    w_qkv = np.asarray(w_qkv, dtype=np.float32)
    w_out = np.asarray(w_out, dtype=np.float32)

    cosT, sinS = _rope_tables()

    perm = np.zeros((P, P), dtype=BF16_NP)
    for i in range(P):
        perm[(i + 64) % P, i] = 1

    mask = np.zeros((P, 4 * 512), dtype=BF16_NP)
    r_idx = np.arange(P)[:, None]
    c_idx = np.arange(512)[None, :]
    for oi, off in enumerate((0, 128, 256, 384)):
        mask[:, oi * 512 : (oi + 1) * 512] = (r_idx + off <= c_idx).astype(BF16_NP)

    # per-batch x^T fp8 hi/lo tiles: [TB, 128, DT, 512]
    xhs, xls = [], []
    for b in range(B):
        xT = np.ascontiguousarray(x[b].T)                          # [D, T]
        hi, lo = _split8(xT, SX)
        xhs.append(
            np.ascontiguousarray(
                hi.reshape(DT, P, TB, 512).transpose(2, 1, 0, 3)
            )
        )
        xls.append(
            np.ascontiguousarray(
                lo.reshape(DT, P, TB, 512).transpose(2, 1, 0, 3)
            )
        )

    # per head-group weight shards (fp8 hi/lo pairs)
    wqkhs, wqkls, wvhs, wvls, wohs, wols = [], [], [], [], [], []
    for g in range(4):
        h0 = NH * g
        cols = [w_qkv[:, 128 * (h0 + h) : 128 * (h0 + h + 1)] for h in range(NH)]
        cols += [
            w_qkv[:, INNER + 128 * (h0 + h) : INNER + 128 * (h0 + h + 1)]
            for h in range(NH)
        ]
        W = np.concatenate(cols, axis=1)                            # [D, 8*128]
        hi, lo = _split8(W, SW)
        for src, dst in ((hi, wqkhs), (lo, wqkls)):
            t_ = src.reshape(DT, P, 8, P).transpose(1, 2, 0, 3).reshape(P, 8 * DT, P)
            dst.append(np.ascontiguousarray(t_))

        WV = w_qkv[:, 2 * INNER + 128 * h0 : 2 * INNER + 128 * (h0 + NH)]  # [D, 512]
        hi, lo = _split8(WV, SW)
        for src, dst in ((hi, wvhs), (lo, wvls)):
            t_ = src.reshape(DT, P, 512).transpose(1, 0, 2)
            dst.append(np.ascontiguousarray(t_))

        WO = w_out[128 * h0 : 128 * (h0 + NH), :]                   # [512, D]
        hi, lo = _split8(WO, SW)
        for src, dst in ((hi, wohs), (lo, wols)):
            t_ = src.reshape(NH, P, DIM).transpose(1, 0, 2)
            dst.append(np.ascontiguousarray(t_))

    if "nc" not in _CACHE:
        _CACHE["nc"] = _build_nc()
    nc = _CACHE["nc"]

    in_maps = []
    for c in range(N_CORES):
        b, g = divmod(c, 4)
        in_maps.append(
            {
                "xh": xhs[b],
                "xl": xls[b],
                "wqkh": wqkhs[g],
                "wqkl": wqkls[g],
                "wvh": wvhs[g],
                "wvl": wvls[g],
                "woh": wohs[g],
                "wol": wols[g],
                "cosT": cosT,
                "sinT": sinS,
                "mask": mask,
                "perm": perm,
            }
        )

    res = run_bass_kernel_spmd(nc, in_maps, core_ids=list(range(N_CORES)))

    out = np.zeros((B, T, DIM), dtype=np.float32)
    for c in range(N_CORES):
        b = c // 4
        out[b] += res.results[c]["out"].astype(np.float32)
    return out
